# revision 23
# baseline (speedup 1.0000x reference)
"""Trainium2 Bass kernel for causal multi-head attention with RoPE.

Full module: qkv = x @ w_qkv; RoPE(q, k); causal softmax attention;
out = attn_out @ w_out.  x: [2, 2048, 1024], 16 heads x 64 dim.

Sharding: 8 cores = 2 batches x 4 head-groups (4 heads/core).  Each core
computes its batch's q/k/v for its heads, runs attention, and produces a
partial [2048, 1024] output through its slice of w_out.  Host sums the 4
partials per batch.

v3 design notes:
- all matmul operands bf16, pre-cast on the host; output partials bf16.
- rotate_half via a PE permutation matmul (no SP-queue DMA swaps); the
  sign lives in the host-built sinT table.
- attention runs on 256-query tiles with [128, 4heads, 256] score tiles
  (one exp per chunk) and both AV accumulator pairs bank-packed via the
  has_written clear-once ordering, so production (2 banks) + scores (4)
  + AV (2) fit in 8 PSUM banks and the phases overlap.
- softmax rowsums ride the ones-column of v; they are gathered across
  heads with tiny selector matmuls on the PE, inverted once per tile on
  DVE, and broadcast back across partitions with indicator matmuls (no
  DRAM round-trip, no SP-queue DMAs on the critical path).
"""

import os
import sys

import numpy as np

for _p in ("/opt/trn_rl_repo", "/root/.axon_site/_ro/trn_rl_repo"):
    if os.path.isdir(_p) and _p not in sys.path:
        sys.path.append(_p)

import concourse.bass as bass
import concourse.mybir as mybir
import concourse.tile as tile

F32 = mybir.dt.float32
F32R = mybir.dt.float32r
BF16 = mybir.dt.bfloat16

# Problem constants (hardcoded per contest rules)
B = 2
N = 2048
D = 1024
HEADS = 16
DH = 64
N_CORES = 8
HL = HEADS // (N_CORES // B)  # heads per core = 4


def build_attention_nc(
    n=N,
    d=D,
    hl=HL,
    dh=DH,
    qt=256,
    pt=512,
    lag=5,
    warm=20,
    cap_waits=True,
):
    """Build the per-core Bass module.  All cores run the same program (SPMD)."""
    nc = bass.Bass("TRN2", target_bir_lowering=False, debug=False)

    KC = d // 128          # contraction chunks for qkv production (8)
    CT = (hl * dh) // 128  # q/k column tiles, 2 heads per tile (2)
    NPT = n // pt          # production seq tiles (4)
    NQT = n // qt          # attention query tiles (8)
    KPQ = qt // 128        # key chunks per query tile (2)
    NSB = n // 128         # seq blocks / key chunks (16)
    OC = (hl * dh) // 128  # w_out contraction chunks from this core (2)
    NOT = d // 512         # output N tiles (2)
    NHP = hl // 2          # head pairs (2)
    scale = float(dh) ** -0.5

    xT = nc.dram_tensor("xT", [d, n], BF16, kind="ExternalInput").ap()
    wq = nc.dram_tensor("wq", [d, hl * dh], BF16, kind="ExternalInput").ap()
    wk = nc.dram_tensor("wk", [d, hl * dh], BF16, kind="ExternalInput").ap()
    wv = nc.dram_tensor("wv", [d, hl * dh], BF16, kind="ExternalInput").ap()
    wo = nc.dram_tensor("wo", [hl * dh, d], BF16, kind="ExternalInput").ap()
    cosT = nc.dram_tensor("cosT", [128, n], BF16, kind="ExternalInput").ap()
    sinT = nc.dram_tensor("sinT", [128, n], BF16, kind="ExternalInput").ap()
    triD = nc.dram_tensor("tri", [128, 128], BF16, kind="ExternalInput").ap()
    permD = nc.dram_tensor("perm", [128, 128], BF16, kind="ExternalInput").ap()
    indD = nc.dram_tensor("ind", [hl, hl, dh], F32, kind="ExternalInput").ap()
    selD = nc.dram_tensor("sel", [dh + 1, hl, hl], F32, kind="ExternalInput").ap()
    outp = nc.dram_tensor("out_partial", [n, d], BF16, kind="ExternalOutput").ap()

    with tile.TileContext(nc) as tc:
        with tc.tile_pool(name="pers", bufs=1) as pers:
            # Persistent SBUF tensors
            qT_sb = [pers.tile([128, n], BF16, tag=f"qT{i}", name=f"qT{i}") for i in range(CT)]
            kT_sb = [pers.tile([128, n], BF16, tag=f"kT{i}", name=f"kT{i}") for i in range(CT)]
            v_sb = [pers.tile([128, hl, dh + 1], BF16, tag=f"v{i}", name=f"v{i}") for i in range(NSB)]
            x_sb = [pers.tile([128, n], BF16, tag=f"x{kc}", name=f"x{kc}") for kc in range(KC)]
            wq_sb = pers.tile([128, KC, hl * dh], BF16, tag="wq", name="wq")
            wk_sb = pers.tile([128, KC, hl * dh], BF16, tag="wk", name="wk")
            wv_sb = pers.tile([128, KC, hl * dh], BF16, tag="wv", name="wv")
            wo_sb = pers.tile([128, OC, d], BF16, tag="wo", name="wo")
            cos_sb = pers.tile([128, n], BF16, tag="cos", name="cos")
            sin_sb = pers.tile([128, n], BF16, tag="sin", name="sin")
            tri_sb = pers.tile([128, 128], BF16, tag="tri", name="tri")
            perm_sb = pers.tile([128, 128], BF16, tag="perm", name="perm")
            indf_sb = pers.tile([hl, hl, dh], F32, tag="indf", name="indf")
            ind_sb = pers.tile([hl, hl, dh], F32R, tag="ind", name="ind")
            self_sb = pers.tile([dh + 1, hl, hl], F32, tag="selft", name="selft")
            sel_sb = pers.tile([dh + 1, hl, hl], F32R, tag="sel", name="sel")
            onec_sb = pers.tile([128, 1], BF16, tag="onec", name="onec")
            outT_sb = [pers.tile([128, n], BF16, tag=f"oT{i}", name=f"oT{i}") for i in range(CT)]

            # ---- input DMAs: ordered so the first production matmuls can
            # start ASAP; large tensors split for multi-queue overlap.
            wqr = wq.rearrange("(kc p) m -> p kc m", p=128)
            wkr = wk.rearrange("(kc p) m -> p kc m", p=128)
            wvr = wv.rearrange("(kc p) m -> p kc m", p=128)
            wor = wo.rearrange("(kc p) m -> p kc m", p=128)
            nc.sync.dma_start(indf_sb, indD)
            nc.sync.dma_start(self_sb, selD)
            nc.sync.dma_start(wq_sb[:, 0:4, :], wqr[:, 0:4, :])
            nc.sync.dma_start(wq_sb[:, 4:8, :], wqr[:, 4:8, :])
            nc.sync.dma_start(x_sb[0][:, 0:pt], xT[0:128, 0:pt])
            nc.sync.dma_start(wk_sb[:, 0:4, :], wkr[:, 0:4, :])
            nc.sync.dma_start(wk_sb[:, 4:8, :], wkr[:, 4:8, :])
            for kc in range(1, KC):
                nc.sync.dma_start(x_sb[kc][:, 0:pt], xT[kc * 128 : (kc + 1) * 128, 0:pt])
            nc.sync.dma_start(cos_sb[:, 0:pt], cosT[:, 0:pt])
            nc.sync.dma_start(sin_sb[:, 0:pt], sinT[:, 0:pt])
            nc.sync.dma_start(perm_sb, permD)
            nc.sync.dma_start(wv_sb[:, 0:4, :], wvr[:, 0:4, :])
            nc.sync.dma_start(wv_sb[:, 4:8, :], wvr[:, 4:8, :])
            nc.sync.dma_start(tri_sb, triD)
            nc.gpsimd.tensor_copy(ind_sb, indf_sb)
            nc.gpsimd.tensor_copy(sel_sb, self_sb)
            for st in range(1, NPT):
                sl = slice(st * pt, (st + 1) * pt)
                for kc in range(KC):
                    nc.sync.dma_start(x_sb[kc][:, sl], xT[kc * 128 : (kc + 1) * 128, sl])
                nc.sync.dma_start(cos_sb[:, sl], cosT[:, sl])
                nc.sync.dma_start(sin_sb[:, sl], sinT[:, sl])
            nc.sync.dma_start(wo_sb[:, 0, :], wor[:, 0, :])
            nc.sync.dma_start(wo_sb[:, 1, :], wor[:, 1, :])

            # ---- Interleaved production + attention.  In-order engines make
            # queue order = execution order, so emit production st-blocks and
            # the attention tiles they enable alternately; v-blocks ride in
            # the attention stream as PE filler.
            with (
                tc.tile_pool(name="psP", bufs=2, space="PSUM") as psP,
                tc.tile_pool(name="ropet", bufs=4) as ropet,
                tc.tile_pool(name="psS", bufs=2, space="PSUM") as psS,
                tc.tile_pool(name="psAV", bufs=1, space="PSUM") as psAV,
                tc.tile_pool(name="expp", bufs=2 * lag + 2) as expp,
                tc.tile_pool(name="up", bufs=4) as up,
                tc.tile_pool(name="ep", bufs=4) as ep,
                tc.tile_pool(name="fo", bufs=4) as fo,
            ):
                # PE warm-up while the first DMAs land
                warm_sb = ropet.tile([128, 512], BF16, tag="warm", name="warm")
                nc.vector.memset(warm_sb, 0.0)
                nc.vector.memset(onec_sb, 1.0)
                wps = psP.tile([128, pt], F32, tag="p", name="wps")
                for i in range(warm):
                    nc.tensor.matmul(
                        wps, warm_sb[:, 0:128], warm_sb, start=(i == 0), stop=(i == warm - 1)
                    )

                def emit_production_qk(st):
                    sl = slice(st * pt, (st + 1) * pt)
                    for w_mm, dst in ((wq_sb, qT_sb), (wk_sb, kT_sb)):
                        for ct in range(CT):
                            ps = psP.tile([128, pt], F32, tag="p", name="p")
                            for kc in range(KC):
                                nc.tensor.matmul(
                                    ps,
                                    w_mm[:, kc, ct * 128 : (ct + 1) * 128],
                                    x_sb[kc][:, sl],
                                    start=(kc == 0),
                                    stop=(kc == KC - 1),
                                )
                            raw_t = ropet.tile([128, pt], BF16, tag="raw", name="raw")
                            a_t = ropet.tile([128, pt], BF16, tag="a", name="a")
                            sh_t = ropet.tile([128, pt], BF16, tag="sh", name="sh")
                            nc.scalar.copy(raw_t, ps)
                            rps = psP.tile([128, pt], F32, tag="p", name="rp")
                            nc.tensor.matmul(rps, perm_sb, raw_t, start=True, stop=True)
                            nc.gpsimd.tensor_tensor(a_t, raw_t, cos_sb[:, sl], mybir.AluOpType.mult)
                            nc.vector.tensor_tensor(sh_t, rps, sin_sb[:, sl], mybir.AluOpType.mult)
                            nc.vector.tensor_tensor(dst[ct][:, sl], a_t, sh_t, mybir.AluOpType.add)

                def emit_v_block(sb):
                    psv = psP.tile([128, pt], F32, tag="p", name="v")
                    for kc in range(KC):
                        nc.tensor.matmul(
                            psv[:, 0 : hl * dh],
                            x_sb[kc][:, sb * 128 : (sb + 1) * 128],
                            wv_sb[:, kc, :],
                            start=(kc == 0),
                            stop=(kc == KC - 1),
                        )
                    nc.vector.tensor_copy(
                        v_sb[sb][:, :, 0:dh],
                        psv[:, 0 : hl * dh].rearrange("p (h e) -> p h e", h=hl),
                    )
                    nc.vector.tensor_copy(
                        v_sb[sb][:, :, dh : dh + 1],
                        onec_sb[:, None, :].to_broadcast([128, hl, 1]),
                    )

                u_tiles = {}

                def emit_sc_av(t):
                    pav = [
                        psAV.tile([dh + 1, 2, qt], F32, tag=f"av{hp}", name=f"av{hp}")
                        for hp in range(NHP)
                    ]
                    ncc = KPQ * (t + 1)
                    e_ts = {}

                    def emit_scores(c, t=t, e_ts=e_ts):
                        j = c - KPQ * t
                        lo = max(0, j * 128)
                        # concurrent row-packed pair (g=0/1) must drain to
                        # different PSUM banks -> g-major region layout
                        pss = psS.tile([128, 2, NHP, qt], F32, tag="s", name="s")
                        for hp in range(NHP):
                            for g in range(2):
                                bp = 64 * g
                                nc.tensor.matmul(
                                    pss[:, g, hp, lo:qt],
                                    kT_sb[hp][bp : bp + dh, c * 128 : (c + 1) * 128],
                                    qT_sb[hp][bp : bp + dh, t * qt + lo : (t + 1) * qt],
                                    start=True,
                                    stop=True,
                                )
                        e_t = expp.tile([128, 2, NHP, qt], BF16, tag="e", name="e")
                        nc.scalar.activation(
                            e_t[:, :, :, lo:qt],
                            pss[:, :, :, lo:qt],
                            mybir.ActivationFunctionType.Exp,
                            scale=scale,
                        )
                        if j >= 0:
                            nc.gpsimd.tensor_tensor(
                                e_t[:, :, :, lo : lo + 128],
                                e_t[:, :, :, lo : lo + 128],
                                tri_sb[:, None, None, :].to_broadcast([128, 2, NHP, 128]),
                                mybir.AluOpType.mult,
                            )
                        e_ts[c] = e_t

                    def emit_av(c, t=t, e_ts=e_ts, pav=pav, ncc=ncc):
                        lo = max(0, (c - KPQ * t) * 128)
                        e_t = e_ts.pop(c)
                        for hp in range(NHP):
                            for g in range(2):
                                h = 2 * hp + g
                                nc.tensor.matmul(
                                    pav[hp][:, g, lo:qt],
                                    v_sb[c][:, h, :],
                                    e_t[:, g, hp, lo:qt],
                                    start=(c == 0 and g == 0),
                                    stop=(c == ncc - 1),
                                    skip_group_check=True,
                                )

                    for c in range(ncc):
                        emit_scores(c)
                        if c >= lag:
                            emit_av(c - lag)
                    for c in range(max(0, ncc - lag), ncc):
                        emit_av(c)

                    # evacuate pav now so the next tile's AV can claim the banks
                    u_t = [
                        up.tile([dh + 1, 2, qt], F32R, tag=f"u{hp}", name=f"u{hp}")
                        for hp in range(NHP)
                    ]
                    with nc.allow_low_precision(reason="f32r attn numerators"):
                        for hp in range(NHP):
                            nc.vector.tensor_copy(u_t[hp], pav[hp])
                    u_tiles[t] = u_t

                def emit_ef(t):
                    qsl = slice(t * qt, (t + 1) * qt)
                    u_t = u_tiles.pop(t)
                    # gather the 4 rowsum rows (partition dh of each u half)
                    # onto partitions 0..3 with selector matmuls, then invert.
                    rsps = psS.tile([128, hl, qt], F32, tag="s", name="rsps")
                    for hp in range(NHP):
                        for g in range(2):
                            h = 2 * hp + g
                            nc.tensor.matmul(
                                rsps[0:hl, 0, :],
                                sel_sb[:, :, h],
                                u_t[hp][:, g, :],
                                start=(h == 0),
                                stop=(h == hl - 1),
                                skip_group_check=True,
                            )
                    rec = ep.tile([hl, qt], F32R, tag="rec", name="rec")
                    with nc.allow_low_precision(reason="f32r 1/rowsum"):
                        nc.vector.reciprocal(rec, rsps[0:hl, 0, :])
                    bps = psS.tile([128, hl, qt], F32, tag="s", name="bps")
                    for h in range(hl):
                        hp, g = h // 2, h % 2
                        bp = 64 * g
                        r = 2 * (h % 2) + h // 2  # alternate banks down the chain
                        nc.tensor.matmul(
                            bps[0:dh, r, :], ind_sb[:, h, :], rec, start=True, stop=True,
                            skip_group_check=True,
                        )
                        nc.vector.tensor_tensor(
                            outT_sb[hp][bp : bp + dh, qsl],
                            u_t[hp][0:dh, g, :],
                            bps[0:dh, r, :],
                            mybir.AluOpType.mult,
                        )

                    # ---- F(t): output projection for this qtile's rows ----
                    for sb in range(t * qt // 128, (t + 1) * qt // 128):
                        o_t = fo.tile([128, NOT, 512], BF16, tag="ot", name="ot")
                        for nt in range(NOT):
                            pso = psS.tile([128, 512], F32, tag="s", name="pso")
                            for kc in range(OC):
                                nc.tensor.matmul(
                                    pso,
                                    outT_sb[kc][:, sb * 128 : (sb + 1) * 128],
                                    wo_sb[:, kc, nt * 512 : (nt + 1) * 512],
                                    start=(kc == 0),
                                    stop=(kc == OC - 1),
                                )
                            nc.vector.tensor_copy(o_t[:, nt, :], pso)
                        nc.sync.dma_start(
                            outp[sb * 128 : (sb + 1) * 128, :],
                            o_t.rearrange("p a b -> p (a b)"),
                        )

                for st in range(NPT):
                    emit_production_qk(st)
                    for t in (2 * st, 2 * st + 1):
                        emit_v_block(2 * t)
                        emit_v_block(2 * t + 1)
                        emit_sc_av(t)
                        if t > 0:
                            emit_ef(t - 1)
                emit_ef(NQT - 1)
    if cap_waits:
        _cap_matmul_waits(nc)
    return nc


_CAPPED_INSTS = {
    "InstMatmult",
    "InstTensorTensor",
    "InstTensorCopy",
    "InstActivation",
    "InstTensorScalarAffineSelect",
    "InstTensorScalar",
    "InstTensorReduce",
    "InstMemset",
    "InstReciprocal",
    "InstLdweights",
    "InstTensorTensorScan",
    "InstIota",
    "InstDMACopy",
    "InstDrain",
}


def _cap_matmul_waits(nc, max_keep=1):
    """Walrus codegen allows only one sync-wait per compute instruction
    (S3 struct wait slots).  Move excess waits onto NoOps inserted just
    before, on the same engine; engines execute in order so the semantics
    are identical."""
    nop_id = 0
    for f in nc.m.functions:
        for blk in f.blocks:
            insts = blk.instructions
            idx = 0
            while idx < len(insts):
                inst = insts[idx]
                if (
                    type(inst).__name__ in _CAPPED_INSTS
                    and inst.sync_info is not None
                    and len(inst.sync_info.on_wait or []) > max_keep
                ):
                    waits = list(inst.sync_info.on_wait)
                    extra, keep = waits[:-max_keep], waits[-max_keep:]
                    inst.sync_info = mybir.SyncInfo(
                        on_wait=keep, on_update=list(inst.sync_info.on_update or [])
                    )
                    for w in extra:
                        nop = mybir.InstNoOp(name=f"I-mmwait-nop-{nop_id}")
                        nop_id += 1
                        nop.engine = inst.engine
                        nop.sync_info = mybir.SyncInfo(on_wait=[w], on_update=[])
                        insts.insert(idx, nop)
                        idx += 1
                idx += 1


def _rope_tables(n, dh, hl):
    """Host-side RoPE tables in transposed, 2-head-stacked, sign-folded form."""
    inv_freq = 1.0 / (10000.0 ** (np.arange(0, dh, 2, dtype=np.float32) / dh))
    t = np.arange(n, dtype=np.float32)
    freqs = np.outer(t, inv_freq).astype(np.float32)  # [n, dh/2]
    emb = np.concatenate([freqs, freqs], axis=-1)  # [n, dh]
    cos = np.cos(emb).astype(np.float32).T  # [dh, n]
    sin = np.sin(emb).astype(np.float32).T
    sin_signed = sin.copy()
    sin_signed[: dh // 2] *= -1.0
    cosT = np.ascontiguousarray(np.tile(cos, (128 // dh, 1)))
    sinT = np.ascontiguousarray(np.tile(sin_signed, (128 // dh, 1)))
    return cosT, sinT


def _host_consts(hl, dh=DH):
    tri = np.triu(np.ones((128, 128), dtype=np.float32))  # keep key<=query
    # rotate_half partition permutation: out[p] = in[swap(p)] per 64-block
    perm = np.zeros((128, 128), dtype=np.float32)
    for blk in range(2):
        o = blk * 64
        for i in range(32):
            perm[o + 32 + i, o + i] = 1.0  # out[o+i] = in[o+32+i]
            perm[o + i, o + 32 + i] = 1.0  # out[o+32+i] = in[o+i]
    # indicator for broadcasting 1/rowsum row h across dh partitions
    ind = np.zeros((hl, hl, dh), dtype=np.float32)
    for h in range(hl):
        ind[h, h, :] = 1.0
    # selector: rowsum row (partition dh of u half) -> partition h
    sel = np.zeros((dh + 1, hl, hl), dtype=np.float32)
    for h in range(hl):
        sel[dh, h, h] = 1.0
    return tri, perm, ind, sel


_NC_CACHE = {}


def kernel(x, w_qkv, w_out):
    return run(x, w_qkv, w_out)[0]


def run(x, w_qkv, w_out, trace=False, build_kwargs=None):
    import ml_dtypes
    from concourse.bass_utils import run_bass_kernel_spmd

    bf16 = ml_dtypes.bfloat16
    x = np.asarray(x, dtype=np.float32)
    w_qkv = np.asarray(w_qkv, dtype=np.float32)
    w_out = np.asarray(w_out, dtype=np.float32)

    cosT, sinT = _rope_tables(N, DH, HL)
    cosT = cosT.astype(bf16)
    sinT = sinT.astype(bf16)
    tri, perm, ind, sel = _host_consts(HL)
    tri = tri.astype(bf16)
    perm = perm.astype(bf16)
    xb = [np.ascontiguousarray(x[b].T).astype(bf16) for b in range(B)]
    in_maps = []
    for core in range(N_CORES):
        b = core // (N_CORES // B)
        g = core % (N_CORES // B)
        cs = slice(g * HL * DH, (g + 1) * HL * DH)
        in_maps.append(
            {
                "xT": xb[b],
                "wq": np.ascontiguousarray(w_qkv[:, cs]).astype(bf16),
                "wk": np.ascontiguousarray(w_qkv[:, D:][:, cs]).astype(bf16),
                "wv": np.ascontiguousarray(w_qkv[:, 2 * D :][:, cs]).astype(bf16),
                "wo": np.ascontiguousarray(w_out[cs, :]).astype(bf16),
                "cosT": cosT,
                "sinT": sinT,
                "tri": tri,
                "perm": perm,
                "ind": ind,
                "sel": sel,
            }
        )

    key = repr(sorted((build_kwargs or {}).items()))
    if key not in _NC_CACHE:
        _NC_CACHE[key] = build_attention_nc(**(build_kwargs or {}))
    nc = _NC_CACHE[key]

    res = run_bass_kernel_spmd(
        nc, in_maps, core_ids=list(range(N_CORES)), trace=trace
    )
    out = np.zeros((B, N, D), dtype=np.float32)
    for core in range(N_CORES):
        out[core // (N_CORES // B)] += res.results[core]["out_partial"].astype(np.float32)
    return out, res


if __name__ == "__main__":
    rng = np.random.default_rng(0)
    x = rng.standard_normal((B, N, D), dtype=np.float32)
    w_qkv = rng.standard_normal((D, 3 * D), dtype=np.float32) * D**-0.5
    w_out = rng.standard_normal((D, D), dtype=np.float32) * D**-0.5
    out = kernel(x, w_qkv, w_out)
    print("out", out.shape, out.dtype, float(np.abs(out).max()))
